# revision 1
# baseline (speedup 1.0000x reference)
"""Depth rasterization (MANO hand z-buffer @ 640x640 -> bilinear 128x128).

Key identities exploited:
  * jax.image.resize(640->128, linear, antialias=False) samples input coords
    5*j + 2.0 exactly -> output[i, j] == raster[5i+2, 5j+2]. Only the 128x128
    decimated pixel grid (centers x = 5j+2.5, y = 5i+2.5) is rasterized: a
    25x reduction vs the reference's 640x640 raster.
  * Edge functions and barycentric depth are affine in pixel coords, so each
    triangle yields four planes over the basis (j, i, 1):
      P_k = OFF - S * sign(area) * e_k     (k = 0,1,2 penalty planes)
      W   = (e0*z0 + e1*z1 + e2*z2) / area (depth plane)
    key(p, f) = max(P0, P1, P2, W) equals the interpolated depth when p is
    inside triangle f and is >= OFF (>> the 100 clamp) outside; the z-buffer
    is zbuf(p) = min(100, min_f key(p, f)).
  * Plane evaluation is a K=9 bf16 matmul (coefficients split into 3 bf16
    limbs; the (j, i, 1) basis is exact in bf16, giving fp32-grade accuracy
    at bf16 PE speed); planes are pair-merged as comp-A = [P0|W] and
    comp-B = [P1|P2] streams evaluated on alternating PE row-groups.
  * Per 16x8-pixel tile, candidates are bbox-filtered and hierarchical-z
    pruned on the host (exact: a candidate whose minimum possible depth over
    the tile exceeds the best fully-covering candidate's maximum depth can
    never win). Tiles are chunked to <=256 candidates per work item (host
    min-merges chunks), items are rank-parity balanced across each batch's
    two cores, and slot capacities are per-rank maxima across all 8 cores -
    exact for any input, no truncation.
  * DVE work per slot is 3 element passes: one wide tensor_tensor max
    (u = max(compA, compB)) and a custom fused DVE op
    (out = max(u_lo, u_hi); accum = min-reduce seeded at 100).

Sharding: 8 cores; each batch element's 128 tiles split across 2 cores.
"""

import numpy as np
import ml_dtypes

import concourse.bacc as bacc
import concourse.mybir as mybir
import concourse.tile as tile
from concourse.bass_utils import run_bass_kernel_spmd

_B, _V, _F = 4, 778, 1538
_H = _W = 128
_TJ, _TI = 16, 8   # tile size in output pixels (x, y)
_NTILE = (_H // _TI) * (_W // _TJ)  # 128 tiles per batch image
_WMAX = 256        # max slot width (pair-merged 2w <= 512 = one PSUM bank)
_OFF = 1000.0      # penalty-plane offset (>> 100 clamp)
_S = 1.0e9         # penalty scale
_BIGC = 1.0e7      # plane constant for padding/invalid
_CLAMP = 100.0
_COVER_MARGIN = 1.0    # e*s margin (e-units) for the full-cover test
_BOUND_MARGIN = 1e-3   # depth margin for the prune bound

_F32 = mybir.dt.float32
_BF16 = mybir.dt.bfloat16
_BF16_NP = ml_dtypes.bfloat16

_NC_CACHE = {}
_OP_CACHE = {}
PROFILE = {}


def _maxpair_minred_op():
    """Custom DVE op: out = max(in0, in1); accum_out = min(out) seeded s0."""
    if "op" in _OP_CACHE:
        return _OP_CACHE["op"]
    import concourse.dve_ops as dve_ops
    from concourse.dve_spec import C0, Spec, Src0, Src1, lower, maxx, minn
    from concourse.dve_table_gen import dve_ver_for
    from concourse.dve_uop import DveOpSpec

    name = "MAXPAIR_MINRED_ANT"
    for op in dve_ops.OPS:
        if op.name == name:
            _OP_CACHE["op"] = op
            return op
    spec = Spec(body=maxx(Src0, Src1), accum=minn, accum_init=C0)
    opcode = dve_ops._CUSTOM_DVE_ROW_BASE + len(dve_ops.OPS)
    assert opcode < 0x20
    dve_ops._SUB_OPCODE_FOR_NAME[name] = opcode
    ver = dve_ver_for("TRN2")
    sha = DveOpSpec(name=name, opcode=opcode, uops=lower(spec, ver=ver),
                    rd1_en=True).sha(ver)
    op = dve_ops.DveOp(name, spec, subdim=False, uops_sha={ver: sha})
    dve_ops.OPS.append(op)
    dve_ops.CUSTOM_DVE_SPECS[name] = spec
    _OP_CACHE["op"] = op
    return op


def _build_nc(caps, groups):
    """caps: per-slot widths w (32-granular, <= _WMAX); groups: ((w, k), ...)
    of consecutive equal-width slots with 2*k*w <= 512 (one PSUM bank)."""
    nslot = len(caps)
    total2 = 2 * int(sum(caps))
    op = _maxpair_minred_op()
    nc = bacc.Bacc("TRN2", target_bir_lowering=False, debug=False, num_devices=8)
    # dense [128, ...] input: pair-merged coef streams (comp-A = [P0|W] limbs
    # at partitions 0-8 & 64-72, comp-B = [P1|P2] at 32-40 & 96-104), then
    # nslot*128 pixel-basis cols at all four row-groups.
    data_d = nc.dram_tensor("data", [128, total2 + nslot * 128], _BF16, kind="ExternalInput")
    out_d = nc.dram_tensor("out", [128, nslot], _F32, kind="ExternalOutput")

    with tile.TileContext(nc) as tc:
        with (
            tc.tile_pool(name="const", bufs=1) as cpool,
            tc.tile_pool(name="scr", bufs=6) as spool,
            tc.tile_pool(name="ps", bufs=8, space="PSUM") as ppool,
        ):
            zmin = cpool.tile([128, nslot], _F32)
            # coef DMA in ~6 chunks at group boundaries; pix in 4 chunks
            goff = [0]
            for w, k in groups:
                goff.append(goff[-1] + 2 * w * k)
            # chunk boundaries (in groups): fine-grained early so the first
            # compute groups start as soon as their data lands
            gb = [0, 1, 2, 4, 6, 9, 13, 18, 24]
            gb = sorted({min(g, len(groups)) for g in gb} | {len(groups)})
            slot_of_group = [0]
            for w, k in groups:
                slot_of_group.append(slot_of_group[-1] + k)
            ctiles = []  # (col range, tile)
            ptiles = []  # (slot range, tile)
            dmas = []
            for i in range(len(gb) - 1):
                c0, c1 = goff[gb[i]], goff[gb[i + 1]]
                s0, s1 = slot_of_group[gb[i]], slot_of_group[gb[i + 1]]
                if c1 > c0:
                    ct = cpool.tile([128, c1 - c0], _BF16, name=f"coef{i}")
                    ctiles.append((c0, c1, ct))
                    dmas.append((ct, data_d.ap()[:, c0:c1]))
                if s1 > s0:
                    pt = cpool.tile([128, (s1 - s0) * 128], _BF16, name=f"pix{i}")
                    ptiles.append((s0, s1, pt))
                    dmas.append((pt, data_d.ap()[:, total2 + s0 * 128 : total2 + s1 * 128]))
            for dst, srcap in dmas:
                nc.sync.dma_start(dst[:], srcap)

            def coef_view(c0, c1):
                for t0, t1, ct in ctiles:
                    if t0 <= c0 and c1 <= t1:
                        return ct[:, c0 - t0 : c1 - t0]
                raise AssertionError((c0, c1))

            def pix_view(s):
                for s0, s1, pt in ptiles:
                    if s0 <= s < s1:
                        return pt[:, (s - s0) * 128 : (s - s0 + 1) * 128]
                raise AssertionError(s)

            gbase = 0
            for gi, (w, k) in enumerate(groups):
                kw2 = 2 * w * k
                go = goff[gi]
                pa = ppool.tile([128, 512], _F32, tag="ps", name="pa")
                pb = ppool.tile([128, 512], _F32, tag="ps", name="pb")
                for q in range(k):
                    s = gbase + q
                    o = 2 * w * q
                    ra, rb = (0, 32) if gi % 2 == 0 else (64, 96)
                    pv = pix_view(s)
                    cv = coef_view(go + o, go + o + 2 * w)
                    nc.tensor.matmul(pa[:, o : o + 2 * w], pv[ra : ra + 9, :],
                                     cv[ra : ra + 9, :],
                                     start=True, stop=True, tile_position=(ra, 0))
                    nc.tensor.matmul(pb[:, o : o + 2 * w], pv[rb : rb + 9, :],
                                     cv[rb : rb + 9, :],
                                     start=True, stop=True, tile_position=(rb, 0))
                # ScalarE pulls comp-A to SBUF (DVE reads max one PSUM operand)
                ta = spool.tile([128, 512], _F32, tag="ta", name="ta")
                nc.scalar.copy(ta[:, :kw2], pa[:, :kw2])
                u = spool.tile([128, 512], _F32, tag="u", name="u")
                nc.vector.tensor_tensor(u[:, :kw2], ta[:, :kw2], pb[:, :kw2],
                                        op=mybir.AluOpType.max)
                for q in range(k):
                    s = gbase + q
                    o = 2 * w * q
                    keyt = spool.tile([128, 256], _F32, tag="key", name="keyt")
                    if PROFILE.get("no_custom"):
                        nc.vector.tensor_tensor(keyt[:, :w], u[:, o : o + w],
                                                u[:, o + w : o + 2 * w],
                                                op=mybir.AluOpType.max)
                        nc.vector.tensor_reduce(zmin[:, s : s + 1], keyt[:, :w],
                                                axis=mybir.AxisListType.X,
                                                op=mybir.AluOpType.min)
                    else:
                        nc.vector._custom_dve(
                            op,
                            out=keyt[:, :w],
                            in0=u[:, o : o + w],
                            in1=u[:, o + w : o + 2 * w],
                            s0=_CLAMP,
                            accum_out=zmin[:, s : s + 1],
                        )
                gbase += k

            nc.sync.dma_start(out_d.ap(), zmin[:])

    nc.compile()
    return nc


def _get_nc(caps, groups):
    key = (caps, groups)
    if key not in _NC_CACHE:
        _NC_CACHE[key] = _build_nc(caps, groups)
    return _NC_CACHE[key]


def _planes64(vertices, faces):
    """Full-precision planes on basis (j, i, 1): [B, 4, 3, F] f64 + aux."""
    v64 = vertices.astype(np.float64)
    fidx = np.asarray(faces).astype(np.int64).reshape(-1)
    fv = v64[:, fidx, :].reshape(_B, _F, 3, 3)
    x0, y0, z0 = fv[:, :, 0, 0], fv[:, :, 0, 1], fv[:, :, 0, 2]
    x1, y1, z1 = fv[:, :, 1, 0], fv[:, :, 1, 1], fv[:, :, 1, 2]
    x2, y2, z2 = fv[:, :, 2, 0], fv[:, :, 2, 1], fv[:, :, 2, 2]

    # area exactly as the reference computes it (float32 ops)
    v32 = vertices.astype(np.float32)
    fv32 = v32[:, fidx, :].reshape(_B, _F, 3, 3)
    xa, ya = fv32[:, :, 0, 0], fv32[:, :, 0, 1]
    xb, yb = fv32[:, :, 1, 0], fv32[:, :, 1, 1]
    xc, yc = fv32[:, :, 2, 0], fv32[:, :, 2, 1]
    area32 = (xb - xa) * (yc - ya) - (yb - ya) * (xc - xa)
    s = np.sign(area32).astype(np.float64)
    valid = np.abs(area32) > 1e-12

    A0 = -(y2 - y1); B0 = x2 - x1; C0 = (y2 - y1) * x1 - (x2 - x1) * y1
    A1 = -(y0 - y2); B1 = x0 - x2; C1 = (y0 - y2) * x2 - (x0 - x2) * y2
    A2 = -(y1 - y0); B2 = x1 - x0; C2 = (y1 - y0) * x0 - (x1 - x0) * y0

    area64 = np.where(valid, area32.astype(np.float64), 1.0)
    Aw = (z0 * A0 + z1 * A1 + z2 * A2) / area64
    Bw = (z0 * B0 + z1 * B1 + z2 * B2) / area64
    Cw = (z0 * C0 + z1 * C1 + z2 * C2) / area64

    planes = np.zeros((_B, 4, 3, _F), np.float64)
    raw = [
        (-_S * s * A0, -_S * s * B0, _OFF - _S * s * C0),
        (-_S * s * A1, -_S * s * B1, _OFF - _S * s * C1),
        (-_S * s * A2, -_S * s * B2, _OFF - _S * s * C2),
        (Aw, Bw, Cw),
    ]
    for k, (a, b, c) in enumerate(raw):
        a = np.where(valid, a, 0.0)
        b = np.where(valid, b, 0.0)
        c = np.where(valid, c, _BIGC)
        # basis change px = 5j + 2.5, py = 5i + 2.5 -> (j, i, 1)
        planes[:, k, 0] = 5.0 * a
        planes[:, k, 1] = 5.0 * b
        planes[:, k, 2] = 2.5 * a + 2.5 * b + c

    xsmin = fv[..., 0].min(2); xsmax = fv[..., 0].max(2)
    ysmin = fv[..., 1].min(2); ysmax = fv[..., 1].max(2)
    zmin_tri = fv[..., 2].min(2)
    return planes, valid, xsmin, xsmax, ysmin, ysmax, zmin_tri


def _split3(c64):
    hi = c64.astype(_BF16_NP).astype(np.float64)
    mid = (c64 - hi).astype(_BF16_NP).astype(np.float64)
    lo = (c64 - hi - mid).astype(_BF16_NP)
    return hi.astype(_BF16_NP), mid.astype(_BF16_NP), lo


def _prepare(vertices, faces):
    planes, valid, xsmin, xsmax, ysmin, ysmax, zmin_tri = _planes64(vertices, faces)
    ntj = _W // _TJ

    # prune per tile, chunk to <=_WMAX, rank-parity balance across all 8
    # cores (a core may hold tiles of any batch - the coef stream is data)
    core_items = [[] for _ in range(8)]  # items: (batch, tile_t, cand_idx_array)
    all_items = []
    for b in range(_B):
        P = planes[b]
        items = all_items
        for t in range(_NTILE):
            tj, ti = t % ntj, t // ntj
            j0, i0 = tj * _TJ, ti * _TI
            xlo, xhi = 5 * j0 + 2.5, 5 * (j0 + _TJ - 1) + 2.5
            ylo, yhi = 5 * i0 + 2.5, 5 * (i0 + _TI - 1) + 2.5
            cand = np.where(valid[b] & (xsmax[b] >= xlo) & (xsmin[b] <= xhi)
                            & (ysmax[b] >= ylo) & (ysmin[b] <= yhi))[0]
            if len(cand):
                corners = np.array(
                    [[j0, i0, 1], [j0 + _TJ - 1, i0, 1],
                     [j0, i0 + _TI - 1, 1], [j0 + _TJ - 1, i0 + _TI - 1, 1]],
                    np.float64)
                Wc = corners @ P[3][:, cand]
                zlo = np.maximum(Wc.min(0), zmin_tri[b][cand])
                covers = np.ones(len(cand), bool)
                for k in range(3):
                    Pc = corners @ P[k][:, cand]
                    covers &= (Pc <= _OFF - _S * _COVER_MARGIN).all(axis=0)
                bound = (Wc.max(0)[covers].min() + _BOUND_MARGIN) if covers.any() else np.inf
                keep = zlo <= bound
                order = cand[keep][np.argsort(zlo[keep])]
            else:
                order = cand
            if len(order) == 0:
                items.append((b, t, order))
            else:
                for c0 in range(0, len(order), _WMAX):
                    items.append((b, t, order[c0 : c0 + _WMAX]))
    all_items.sort(key=lambda it: -len(it[2]))
    for r, it in enumerate(all_items):
        core_items[r % 8].append(it)

    nslot = max(len(ci) for ci in core_items)
    rawcaps = []
    for s in range(nslot):
        m = max((len(ci[s][2]) if s < len(ci) else 0) for ci in core_items)
        rawcaps.append(max(16, ((m + 15) // 16) * 16))

    # groups of consecutive slots padded to the group's (max) width, with
    # pair-merged group width 2*k*w <= 512 (one PSUM bank)
    groups = []
    s = 0
    while s < nslot:
        w = rawcaps[s]
        k = 1
        while s + k < nslot and 2 * (k + 1) * w <= 512:
            k += 1
        groups.append((w, k))
        s += k
    groups = tuple(groups)
    caps = []
    for w, k in groups:
        caps.extend([w] * k)
    caps = tuple(caps)
    total2 = 2 * sum(caps)

    in_maps = []
    for c in range(8):
        items = core_items[c]
        compA = np.zeros((3, total2), np.float64)
        compB = np.zeros((3, total2), np.float64)
        compA[2, :] = _BIGC
        compB[2, :] = _BIGC
        pix_g = np.zeros((3, nslot * 128), np.float32)
        off = 0
        for s in range(nslot):
            w = caps[s]
            jj = ii = np.zeros(128, np.float32)
            if s < len(items):
                b, t, idx = items[s]
                n = len(idx)
                compA[:, off : off + n] = planes[b, 0][:, idx]          # P0
                compA[:, off + w : off + w + n] = planes[b, 3][:, idx]  # W
                compB[:, off : off + n] = planes[b, 1][:, idx]          # P1
                compB[:, off + w : off + w + n] = planes[b, 2][:, idx]  # P2
                tj, ti = t % ntj, t // ntj
                j0, i0 = tj * _TJ, ti * _TI
                jj = j0 + np.tile(np.arange(_TJ, dtype=np.float32), _TI)
                ii = i0 + np.repeat(np.arange(_TI, dtype=np.float32), _TJ)
            off += 2 * w
            pix_g[0, s * 128 : (s + 1) * 128] = jj
            pix_g[1, s * 128 : (s + 1) * 128] = ii
            pix_g[2, s * 128 : (s + 1) * 128] = 1.0
        data = np.zeros((128, total2 + nslot * 128), _BF16_NP)
        for comp, bases in ((compA, (0, 64)), (compB, (32, 96))):
            hi, mid, lo = _split3(comp)
            for base in bases:
                data[base + 0 : base + 3, :total2] = hi
                data[base + 3 : base + 6, :total2] = mid
                data[base + 6 : base + 9, :total2] = lo
        pix16 = np.vstack([pix_g, pix_g, pix_g]).astype(_BF16_NP)
        for base in (0, 32, 64, 96):
            data[base : base + 9, total2:] = pix16
        in_maps.append({"data": data})
    return caps, groups, in_maps, core_items


def kernel(vertices, faces):
    vertices = np.asarray(vertices)
    faces = np.asarray(faces)
    caps, groups, in_maps, core_items = _prepare(vertices, faces)

    nc = _get_nc(caps, groups)
    kw = dict(PROFILE.get("run_kwargs", {}))
    res = run_bass_kernel_spmd(nc, in_maps, list(range(8)), **kw)
    PROFILE["last_result"] = res

    ntj = _W // _TJ
    out = np.full((_B, _H, _W), _CLAMP, np.float32)
    for c in range(8):
        z = res.results[c]["out"]  # [128, nslot]
        for s, (b, t, idx) in enumerate(core_items[c]):
            tj, ti = t % ntj, t // ntj
            j0, i0 = tj * _TJ, ti * _TI
            blk = z[:, s].reshape(_TI, _TJ)
            out[b, i0 : i0 + _TI, j0 : j0 + _TJ] = np.minimum(
                out[b, i0 : i0 + _TI, j0 : j0 + _TJ], blk)
    return out



# revision 10
# speedup vs baseline: 2.1720x; 2.1720x over previous
"""Depth rasterization (MANO hand z-buffer @ 640x640 -> bilinear 128x128).

Key identities:
  * resize(640->128, linear, antialias=False) samples exactly the decimated
    grid: output[i, j] == raster[5i+2, 5j+2] -> rasterize only 128x128 pixels.
  * Per triangle, edge functions / barycentric depth are affine planes over
    pixel coords. key(p, f) = max(P_binding..., W) equals interpolated depth
    inside the triangle and is >= OFF (>> the 100 clamp) outside;
    zbuf(p) = min(100, min_f key(p, f)).
  * Exact per-tile (16x8 px) pruning on the host: bbox overlap + SAT
    (separating-axis: a candidate with all 4 tile corners outside one edge
    never touches the tile) + hierarchical-z (a candidate whose min possible
    depth exceeds the best fully-covering candidate's max depth never wins).
  * Per (candidate, tile), only BINDING edges are streamed: an edge whose
    half-plane contains the whole tile (by exact corner test) can never be
    the max -> candidates carry 1 + #binding planes (avg ~2.5, not 4).
  * Tile-local basis (dj, di, 1), dj=px%16, di=px//16: the tile offset is
    folded into the plane constant on the host, so ONE global 9x128 bf16
    weight block (3 bf16 coefficient limbs x 3 basis rows -> fp32-grade
    coefficients at bf16 PE speed) serves every matmul. Class streams
    (arity 2/3/4) chop into arbitrary 512-col PSUM banks.
  * Per 2-bank PSUM group: matmul -> drain (Act/Pool, fp32->bf16) ->
    tensor_tensor max merges (DVE bf16 runs at 2x; ops statically
    load-balanced across Act/Pool/DVE) -> bf16 key columns -> DRAM.
  * Host does the per-tile min-reduce over candidate key columns + clamp
    (gather/scatter-heavy, trivial in numpy).

Sharding: tiles are greedily balanced across the 8 cores by plane count.
"""

import numpy as np
import ml_dtypes

import concourse.bacc as bacc
import concourse.mybir as mybir
import concourse.tile as tile
from concourse.bass_utils import run_bass_kernel_spmd

_B, _V, _F = 4, 778, 1538
_H = _W = 128
_TJ, _TI = 16, 8
_NTILE = (_H // _TI) * (_W // _TJ)
_OFF = 1000.0
_S = 1.0e9
_BIGC = 1.0e7
_CLAMP = 100.0
_EPS_SAT = 1.0     # e-unit margin: drop only if all corners are outside by > this
_EPS_BIND = 1.0    # e-unit margin: an edge binds unless the whole tile is inside by > this
_BOUND_MARGIN = 1e-3

_F32 = mybir.dt.float32
_BF16 = mybir.dt.bfloat16
_BF16_NP = ml_dtypes.bfloat16

_NC_CACHE = {}
PROFILE = {}

# per-class candidates per 2-bank (1024 fp32 col) PSUM group
_VCLASS = {2: 512, 3: 341, 4: 256}

# static-schedule cost model (ns). GPSIMD(Pool) cannot access PSUM; only
# Act / DVE / DMA drain PSUM. Pool handles SBUF-only merges.
_ACT_COL, _ACT_FIX = 0.833, 210.0
_POOL_COL, _POOL_FIX = 1.39, 130.0
_DVE_BF, _DVE_PS = 0.52, 1.04
_DVE_FIX_SB, _DVE_FIX_PS = 80.0, 190.0
_DMA_COL = 1.43       # 128 part x 4B / 360 B/ns
_DMA_BASE = 3500.0    # in/out stream DMA load (ns)


def _planes64(vertices, faces):
    """Planes on global basis (j, i, 1) (pixel grid coords): [B, 4, 3, F] f64.
    k=0..2: P_k = OFF - S*sign(area)*e_k ; k=3: barycentric depth W."""
    v64 = vertices.astype(np.float64)
    fidx = np.asarray(faces).astype(np.int64).reshape(-1)
    fv = v64[:, fidx, :].reshape(_B, _F, 3, 3)
    x0, y0, z0 = fv[:, :, 0, 0], fv[:, :, 0, 1], fv[:, :, 0, 2]
    x1, y1, z1 = fv[:, :, 1, 0], fv[:, :, 1, 1], fv[:, :, 1, 2]
    x2, y2, z2 = fv[:, :, 2, 0], fv[:, :, 2, 1], fv[:, :, 2, 2]

    # area exactly as the reference computes it (float32 ops)
    v32 = vertices.astype(np.float32)
    fv32 = v32[:, fidx, :].reshape(_B, _F, 3, 3)
    xa, ya = fv32[:, :, 0, 0], fv32[:, :, 0, 1]
    xb, yb = fv32[:, :, 1, 0], fv32[:, :, 1, 1]
    xc, yc = fv32[:, :, 2, 0], fv32[:, :, 2, 1]
    area32 = (xb - xa) * (yc - ya) - (yb - ya) * (xc - xa)
    s = np.sign(area32).astype(np.float64)
    valid = np.abs(area32) > 1e-12

    A0 = -(y2 - y1); B0 = x2 - x1; C0 = (y2 - y1) * x1 - (x2 - x1) * y1
    A1 = -(y0 - y2); B1 = x0 - x2; C1 = (y0 - y2) * x2 - (x0 - x2) * y2
    A2 = -(y1 - y0); B2 = x1 - x0; C2 = (y1 - y0) * x0 - (x1 - x0) * y0

    area64 = np.where(valid, area32.astype(np.float64), 1.0)
    Aw = (z0 * A0 + z1 * A1 + z2 * A2) / area64
    Bw = (z0 * B0 + z1 * B1 + z2 * B2) / area64
    Cw = (z0 * C0 + z1 * C1 + z2 * C2) / area64

    planes = np.zeros((_B, 4, 3, _F), np.float64)
    raw = [
        (-_S * s * A0, -_S * s * B0, _OFF - _S * s * C0),
        (-_S * s * A1, -_S * s * B1, _OFF - _S * s * C1),
        (-_S * s * A2, -_S * s * B2, _OFF - _S * s * C2),
        (Aw, Bw, Cw),
    ]
    for k, (a, b, c) in enumerate(raw):
        a = np.where(valid, a, 0.0)
        b = np.where(valid, b, 0.0)
        c = np.where(valid, c, _BIGC)
        # px = 5j + 2.5, py = 5i + 2.5 -> basis (j, i, 1)
        planes[:, k, 0] = 5.0 * a
        planes[:, k, 1] = 5.0 * b
        planes[:, k, 2] = 2.5 * a + 2.5 * b + c

    xsmin = fv[..., 0].min(2); xsmax = fv[..., 0].max(2)
    ysmin = fv[..., 1].min(2); ysmax = fv[..., 1].max(2)
    zmin_tri = fv[..., 2].min(2)
    return planes, valid, xsmin, xsmax, ysmin, ysmax, zmin_tri


def _split3(c64):
    hi = c64.astype(_BF16_NP).astype(np.float64)
    mid = (c64 - hi).astype(_BF16_NP).astype(np.float64)
    lo = (c64 - hi - mid).astype(_BF16_NP)
    return hi.astype(_BF16_NP), mid.astype(_BF16_NP), lo


def _tiles(vertices, faces):
    """Per (b, t): pruned candidates split by arity class.
    Returns list of dicts with per-class (cand_idx, edges[list per cand])."""
    planes, valid, xsmin, xsmax, ysmin, ysmax, zmin_tri = _planes64(vertices, faces)
    ntj = _W // _TJ
    tiles = []
    for b in range(_B):
        P = planes[b]
        for t in range(_NTILE):
            tj, ti = t % ntj, t // ntj
            j0, i0 = tj * _TJ, ti * _TI
            xlo, xhi = 5 * j0 + 2.5, 5 * (j0 + _TJ - 1) + 2.5
            ylo, yhi = 5 * i0 + 2.5, 5 * (i0 + _TI - 1) + 2.5
            cand = np.where(valid[b] & (xsmax[b] >= xlo) & (xsmin[b] <= xhi)
                            & (ysmax[b] >= ylo) & (ysmin[b] <= yhi))[0]
            ent = {"b": b, "t": t, "j0": j0, "i0": i0,
                   2: (np.empty(0, np.int64), np.empty((0, 1), np.int64)),
                   3: (np.empty(0, np.int64), np.empty((0, 2), np.int64)),
                   4: (np.empty(0, np.int64), np.empty((0, 3), np.int64))}
            if len(cand):
                corners = np.array(
                    [[j0, i0, 1], [j0 + _TJ - 1, i0, 1],
                     [j0, i0 + _TI - 1, 1], [j0 + _TJ - 1, i0 + _TI - 1, 1]],
                    np.float64)
                se = np.stack([(_OFF - corners @ P[k][:, cand]) / _S
                               for k in range(3)])  # [3, 4, n]
                sat_out = (se <= -_EPS_SAT).all(axis=1).any(axis=0)
                binding = se.min(axis=1) < _EPS_BIND  # [3, n]
                covers = (~binding).all(axis=0) & ~sat_out
                Wc = corners @ P[3][:, cand]
                zlo = np.maximum(Wc.min(0), zmin_tri[b][cand])
                bound = (Wc.max(0)[covers].min() + _BOUND_MARGIN) if covers.any() else np.inf
                keep = ~sat_out & (zlo <= bound)
                kidx = np.where(keep)[0]
                nb = binding[:, kidx].sum(axis=0)
                for cls, nbv in ((2, (0, 1)), (3, (2,)), (4, (3,))):
                    m = np.isin(nb, nbv)
                    ci = kidx[m]
                    edges = np.full((len(ci), cls - 1), -1, np.int64)
                    for r, cix in enumerate(ci):
                        eks = np.where(binding[:, cix])[0]
                        edges[r, :len(eks)] = eks
                    ent[cls] = (cand[ci], edges)
            tiles.append(ent)
    return tiles, planes


def _schedule(L):
    """Build shared group structure + static engine schedule from per-class
    stream lengths L = {2: L2, 3: L3, 4: L4}. Returns ordered group dicts."""
    raw = []
    for cls in (2, 3, 4):
        Vc = _VCLASS[cls]
        ngc = (L[cls] + Vc - 1) // Vc
        for g in range(ngc):
            V = min(Vc, L[cls] - g * Vc)
            raw.append((g / max(ngc, 1), cls, g * Vc, V))
    raw.sort()

    busy = {"act": 0.0, "pool": 0.0, "dve": 0.0, "dma": _DMA_BASE}
    groups = []
    co, ko = 0, 0
    for _, cls, soff, V in raw:
        a = cls
        best = None
        # configs: (drain, d). All merges on DVE (Pool has no elementwise
        # ISA on TRN2; only Act/DVE can read PSUM).
        cfgs = [(dr, d, "dve") for dr in ("act", "dve")
                for d in range(1, a + 1)]
        for drain, d, merge in cfgs:
            nb = dict(busy)
            if drain == "act":
                nb["act"] += d * V * _ACT_COL + _ACT_FIX
            elif drain == "dve":
                nb["dve"] += d * V * _DVE_PS + _DVE_FIX_PS
            else:
                nb["dma"] += a * V * _DMA_COL
            fp32 = drain == "dma"
            for lvl in range(1, a):
                if lvl >= d:  # operand still in PSUM -> DVE only
                    nb["dve"] += V * _DVE_PS + _DVE_FIX_PS
                elif merge == "dve":
                    nb["dve"] += V * (_DVE_PS if fp32 else _DVE_BF) + _DVE_FIX_SB
                else:
                    nb["pool"] += V * _POOL_COL + _POOL_FIX
            mx = max(nb.values())
            if best is None or mx < best[0]:
                best = (mx, d, drain, merge, nb)
        _, d, drain, merge, nb = best
        busy = nb
        groups.append({"cls": cls, "soff": soff, "V": V, "d": d,
                       "drain": drain, "merge": merge, "coff": co, "koff": ko})
        co += a * V
        ko += V
    return tuple((g["cls"], g["soff"], g["V"], g["d"], g["drain"], g["merge"],
                  g["coff"], g["koff"]) for g in groups), co, ko


def _build_nc(gkey):
    groups, CT, KT = gkey
    nc = bacc.Bacc("TRN2", target_bir_lowering=False, debug=False, num_devices=8)
    pix_d = nc.dram_tensor("pix", [9, 128], _BF16, kind="ExternalInput")
    coef_d = nc.dram_tensor("coef", [9, CT], _BF16, kind="ExternalInput")
    out_d = nc.dram_tensor("out", [128, KT], _BF16, kind="ExternalOutput")

    with tile.TileContext(nc) as tc:
        with (
            tc.tile_pool(name="const", bufs=1) as cpool,
            tc.tile_pool(name="scr", bufs=4) as spool,
            tc.tile_pool(name="ps", bufs=4, space="PSUM") as ppool,
        ):
            pixt = cpool.tile([9, 128], _BF16, name="pix")
            nc.sync.dma_start(pixt[:], pix_d.ap())
            coeft = cpool.tile([9, CT], _BF16, name="coef")
            # coef DMA in 3 chunks at group boundaries so matmuls start early
            ng = len(groups)
            cuts = sorted({groups[min(i * ng // 3, ng - 1)][6] for i in range(1, 3)}
                          | {0, CT})
            for c0, c1 in zip(cuts[:-1], cuts[1:]):
                if c1 > c0:
                    nc.sync.dma_start(coeft[:, c0:c1], coef_d.ap()[:, c0:c1])
            outt = cpool.tile([128, KT], _BF16, name="out")

            odma = [0]

            def flush_out(k1):
                if k1 > odma[0]:
                    nc.sync.dma_start(out_d.ap()[:, odma[0]:k1], outt[:, odma[0]:k1])
                    odma[0] = k1

            for gi, (cls, soff, V, d, drain, merge, coff, koff) in enumerate(groups):
                a = cls
                aV = a * V
                ps = ppool.tile([128, 1024], _F32, tag="ps", name=f"ps{gi}")
                # bank-aligned matmul chops of [0, aV)
                c0 = 0
                while c0 < aV:
                    c1 = min(aV, (c0 // 512 + 1) * 512)
                    nc.tensor.matmul(ps[:, c0:c1], pixt[:, :],
                                     coeft[:, coff + c0:coff + c1],
                                     start=True, stop=True)
                    c0 = c1
                sc = spool.tile([128, 1024], _BF16, tag="sc", name=f"sc{gi}")
                if drain == "act":
                    nc.scalar.copy(sc[:, :d * V], ps[:, :d * V])
                else:
                    nc.vector.tensor_scalar(sc[:, :d * V], ps[:, :d * V],
                                            0.0, None,
                                            op0=mybir.AluOpType.add)
                cur = sc[:, 0:V]
                for lvl in range(1, a):
                    dst = outt[:, koff:koff + V]
                    in1 = (sc[:, lvl * V:(lvl + 1) * V] if lvl < d
                           else ps[:, lvl * V:(lvl + 1) * V])
                    nc.vector.tensor_tensor(dst, cur, in1,
                                            op=mybir.AluOpType.max)
                    cur = dst
                if (gi + 1) % max(1, len(groups) // 3) == 0:
                    flush_out(koff + V)
            flush_out(KT)

    nc.compile()
    return nc


def _get_nc(gkey):
    if gkey not in _NC_CACHE:
        _NC_CACHE[gkey] = _build_nc(gkey)
    return _NC_CACHE[gkey]


def _prepare(vertices, faces):
    tiles, planes = _tiles(vertices, faces)

    # greedy tile -> core assignment balanced by plane count
    def tplanes(ent):
        return (2 * len(ent[2][0]) + 3 * len(ent[3][0]) + 4 * len(ent[4][0]))

    order = sorted(range(len(tiles)), key=lambda i: -tplanes(tiles[i]))
    loads = [0.0] * 8
    core_tiles = [[] for _ in range(8)]
    for i in order:
        c = loads.index(min(loads))
        core_tiles[c].append(tiles[i])
        loads[c] += tplanes(tiles[i])

    # per-core class streams; runs for host unpacking
    streams = [{2: [], 3: [], 4: []} for _ in range(8)]  # (b_idx, cand, edges, j0, i0)
    runs = [[] for _ in range(8)]  # (cls, spos, n, b, t)
    for c in range(8):
        for ent in core_tiles[c]:
            for cls in (2, 3, 4):
                ci, edges = ent[cls]
                if len(ci) == 0:
                    continue
                runs[c].append((cls, len(streams[c][cls]), len(ci),
                                ent["b"], ent["t"]))
                for r in range(len(ci)):
                    streams[c][cls].append((ent["b"], ci[r], edges[r],
                                            ent["j0"], ent["i0"]))

    L = {cls: max(len(streams[c][cls]) for c in range(8)) for cls in (2, 3, 4)}
    gkey = _schedule(L)
    groups, CT, KT = gkey

    # vectorized coef construction per core
    in_maps = []
    dj = (np.arange(128) % _TJ).astype(np.float64)
    di = (np.arange(128) // _TJ).astype(np.float64)
    pix9 = np.zeros((9, 128), _BF16_NP)
    for r in range(3):
        pix9[3 * r + 0] = dj.astype(_BF16_NP)
        pix9[3 * r + 1] = di.astype(_BF16_NP)
        pix9[3 * r + 2] = 1.0
    for c in range(8):
        coef = np.zeros((9, CT), _BF16_NP)
        for cls in (2, 3, 4):
            st = streams[c][cls]
            n = len(st)
            if n == 0:
                continue
            bv = np.array([s[0] for s in st])
            cv = np.array([s[1] for s in st])
            ev = np.array([s[2] for s in st])  # [n, cls-1]
            j0v = np.array([s[3] for s in st], np.float64)
            i0v = np.array([s[4] for s in st], np.float64)
            for lvl in range(cls):
                if lvl == 0:
                    sel = np.full(n, 3)
                    use = np.ones(n, bool)
                else:
                    sel = ev[:, lvl - 1]
                    use = sel >= 0
                    sel = np.where(use, sel, 0)
                al = planes[bv, sel, 0, cv]
                be = planes[bv, sel, 1, cv]
                ga = planes[bv, sel, 2, cv] + al * j0v + be * i0v
                al = np.where(use, al, 0.0)
                be = np.where(use, be, 0.0)
                ga = np.where(use, ga, -_BIGC)
                block = np.empty((9, n), _BF16_NP)
                h_a, m_a, l_a = _split3(al)
                h_b, m_b, l_b = _split3(be)
                h_c, m_c, l_c = _split3(ga)
                block[0], block[1], block[2] = h_a, h_b, h_c
                block[3], block[4], block[5] = m_a, m_b, m_c
                block[6], block[7], block[8] = l_a, l_b, l_c
                # scatter into groups of this class
                for (gcls, soff, V, d, drain, merge, coff, koff) in groups:
                    if gcls != cls:
                        continue
                    s0, s1 = soff, min(soff + V, n)
                    if s1 <= s0:
                        continue
                    coef[:, coff + lvl * V + 0: coff + lvl * V + (s1 - s0)] = \
                        block[:, s0:s1]
        in_maps.append({"coef": coef, "pix": pix9})

    return gkey, in_maps, runs


def _unpack(gkey, results, runs):
    groups, CT, KT = gkey
    # per class: list of (soff, V, koff) for stream-pos -> out-col mapping
    gmap = {2: [], 3: [], 4: []}
    for (cls, soff, V, d, drain, merge, coff, koff) in groups:
        gmap[cls].append((soff, V, koff))
    ntj = _W // _TJ
    out = np.full((_B, _H, _W), _CLAMP, np.float32)
    for c in range(8):
        key = results[c]["out"].astype(np.float32)  # [128, KT]
        for (cls, spos, n, b, t) in runs[c]:
            tj, ti = t % ntj, t // ntj
            j0, i0 = tj * _TJ, ti * _TI
            vals = None
            s0 = spos
            while s0 < spos + n:
                for (soff, V, koff) in gmap[cls]:
                    if soff <= s0 < soff + V:
                        s1 = min(spos + n, soff + V)
                        seg = key[:, koff + (s0 - soff): koff + (s1 - soff)]
                        m = seg.min(axis=1)
                        vals = m if vals is None else np.minimum(vals, m)
                        s0 = s1
                        break
                else:
                    raise AssertionError((cls, s0))
            blk = vals.reshape(_TI, _TJ)
            np.minimum(out[b, i0:i0 + _TI, j0:j0 + _TJ], blk,
                       out=out[b, i0:i0 + _TI, j0:j0 + _TJ])
    return out


def kernel(vertices, faces):
    vertices = np.asarray(vertices)
    faces = np.asarray(faces)
    gkey, in_maps, runs = _prepare(vertices, faces)
    nc = _get_nc(gkey)
    kw = dict(PROFILE.get("run_kwargs", {}))
    res = run_bass_kernel_spmd(nc, in_maps, list(range(8)), **kw)
    PROFILE["last_result"] = res
    return _unpack(gkey, res.results, runs)


# revision 13
# speedup vs baseline: 2.1967x; 1.0114x over previous
"""Depth rasterization (MANO hand z-buffer @ 640x640 -> bilinear 128x128).

Key identities:
  * resize(640->128, linear, antialias=False) samples exactly the decimated
    grid: output[i, j] == raster[5i+2, 5j+2] -> rasterize only 128x128 pixels.
  * Per triangle, edge functions / barycentric depth are affine planes over
    pixel coords. key(p, f) = max(P_binding..., W) equals interpolated depth
    inside the triangle and is >= OFF (>> the 100 clamp) outside;
    zbuf(p) = min(100, min_f key(p, f)).
  * Exact per-tile (16x8 px) pruning on the host: bbox overlap + SAT
    (separating-axis: a candidate with all 4 tile corners outside one edge
    never touches the tile) + hierarchical-z (a candidate whose min possible
    depth exceeds the best fully-covering candidate's max depth never wins).
  * Per (candidate, tile), only BINDING edges are streamed: an edge whose
    half-plane contains the whole tile (by exact corner test) can never be
    the max -> candidates carry 1 + #binding planes (avg ~2.5, not 4).
  * Tile-local basis (dj, di, 1), dj=px%16, di=px//16: the tile offset is
    folded into the plane constant on the host, so ONE global 9x128 bf16
    weight block (3 bf16 coefficient limbs x 3 basis rows -> fp32-grade
    coefficients at bf16 PE speed) serves every matmul. Class streams
    (arity 2/3/4) chop into arbitrary 512-col PSUM banks.
  * Per 2-bank PSUM group: matmul -> drain (Act/Pool, fp32->bf16) ->
    tensor_tensor max merges (DVE bf16 runs at 2x; ops statically
    load-balanced across Act/Pool/DVE) -> bf16 key columns -> DRAM.
  * Host does the per-tile min-reduce over candidate key columns + clamp
    (gather/scatter-heavy, trivial in numpy).

Sharding: tiles are greedily balanced across the 8 cores by plane count.
"""

import numpy as np
import ml_dtypes

import concourse.bacc as bacc
import concourse.mybir as mybir
import concourse.tile as tile
from concourse.bass_utils import run_bass_kernel_spmd

_B, _V, _F = 4, 778, 1538
_H = _W = 128
_TJ, _TI = 16, 8
_NTILE = (_H // _TI) * (_W // _TJ)
_OFF = 1000.0
_S = 1.0e9
_BIGC = 1.0e7
_CLAMP = 100.0
_EPS_SAT = 1.0     # e-unit margin: drop only if all corners are outside by > this
_EPS_BIND = 1.0    # e-unit margin: an edge binds unless the whole tile is inside by > this
_BOUND_MARGIN = 1e-3

_F32 = mybir.dt.float32
_BF16 = mybir.dt.bfloat16
_BF16_NP = ml_dtypes.bfloat16

_NC_CACHE = {}
PROFILE = {}

# per-class candidates per 2-bank (1024 fp32 col) PSUM group
_VCLASS = {2: 512, 3: 341, 4: 256}

# static-schedule cost model (ns). GPSIMD(Pool) cannot access PSUM; only
# Act / DVE / DMA drain PSUM. Pool handles SBUF-only merges.
_ACT_COL, _ACT_FIX = 0.833, 210.0
_POOL_COL, _POOL_FIX = 1.39, 130.0
_DVE_BF, _DVE_PS = 0.52, 1.04
_DVE_FIX_SB, _DVE_FIX_PS = 80.0, 190.0
_DMA_COL = 1.43       # 128 part x 4B / 360 B/ns
_DMA_BASE = 3500.0    # in/out stream DMA load (ns)


def _planes64(vertices, faces):
    """Planes on global basis (j, i, 1) (pixel grid coords): [B, 4, 3, F] f64.
    k=0..2: P_k = OFF - S*sign(area)*e_k ; k=3: barycentric depth W."""
    v64 = vertices.astype(np.float64)
    fidx = np.asarray(faces).astype(np.int64).reshape(-1)
    fv = v64[:, fidx, :].reshape(_B, _F, 3, 3)
    x0, y0, z0 = fv[:, :, 0, 0], fv[:, :, 0, 1], fv[:, :, 0, 2]
    x1, y1, z1 = fv[:, :, 1, 0], fv[:, :, 1, 1], fv[:, :, 1, 2]
    x2, y2, z2 = fv[:, :, 2, 0], fv[:, :, 2, 1], fv[:, :, 2, 2]

    # area exactly as the reference computes it (float32 ops)
    v32 = vertices.astype(np.float32)
    fv32 = v32[:, fidx, :].reshape(_B, _F, 3, 3)
    xa, ya = fv32[:, :, 0, 0], fv32[:, :, 0, 1]
    xb, yb = fv32[:, :, 1, 0], fv32[:, :, 1, 1]
    xc, yc = fv32[:, :, 2, 0], fv32[:, :, 2, 1]
    area32 = (xb - xa) * (yc - ya) - (yb - ya) * (xc - xa)
    s = np.sign(area32).astype(np.float64)
    valid = np.abs(area32) > 1e-12

    A0 = -(y2 - y1); B0 = x2 - x1; C0 = (y2 - y1) * x1 - (x2 - x1) * y1
    A1 = -(y0 - y2); B1 = x0 - x2; C1 = (y0 - y2) * x2 - (x0 - x2) * y2
    A2 = -(y1 - y0); B2 = x1 - x0; C2 = (y1 - y0) * x0 - (x1 - x0) * y0

    area64 = np.where(valid, area32.astype(np.float64), 1.0)
    Aw = (z0 * A0 + z1 * A1 + z2 * A2) / area64
    Bw = (z0 * B0 + z1 * B1 + z2 * B2) / area64
    Cw = (z0 * C0 + z1 * C1 + z2 * C2) / area64

    planes = np.zeros((_B, 4, 3, _F), np.float64)
    raw = [
        (-_S * s * A0, -_S * s * B0, _OFF - _S * s * C0),
        (-_S * s * A1, -_S * s * B1, _OFF - _S * s * C1),
        (-_S * s * A2, -_S * s * B2, _OFF - _S * s * C2),
        (Aw, Bw, Cw),
    ]
    for k, (a, b, c) in enumerate(raw):
        a = np.where(valid, a, 0.0)
        b = np.where(valid, b, 0.0)
        c = np.where(valid, c, _BIGC)
        # px = 5j + 2.5, py = 5i + 2.5 -> basis (j, i, 1)
        planes[:, k, 0] = 5.0 * a
        planes[:, k, 1] = 5.0 * b
        planes[:, k, 2] = 2.5 * a + 2.5 * b + c

    xsmin = fv[..., 0].min(2); xsmax = fv[..., 0].max(2)
    ysmin = fv[..., 1].min(2); ysmax = fv[..., 1].max(2)
    zmin_tri = fv[..., 2].min(2)
    return planes, valid, xsmin, xsmax, ysmin, ysmax, zmin_tri


def _split3(c64):
    hi = c64.astype(_BF16_NP).astype(np.float64)
    mid = (c64 - hi).astype(_BF16_NP).astype(np.float64)
    lo = (c64 - hi - mid).astype(_BF16_NP)
    return hi.astype(_BF16_NP), mid.astype(_BF16_NP), lo


def _tiles(vertices, faces):
    """Per (b, t): pruned candidates split by arity class.
    Returns list of dicts with per-class (cand_idx, edges[list per cand])."""
    planes, valid, xsmin, xsmax, ysmin, ysmax, zmin_tri = _planes64(vertices, faces)
    ntj = _W // _TJ
    tiles = []
    for b in range(_B):
        P = planes[b]
        for t in range(_NTILE):
            tj, ti = t % ntj, t // ntj
            j0, i0 = tj * _TJ, ti * _TI
            xlo, xhi = 5 * j0 + 2.5, 5 * (j0 + _TJ - 1) + 2.5
            ylo, yhi = 5 * i0 + 2.5, 5 * (i0 + _TI - 1) + 2.5
            cand = np.where(valid[b] & (xsmax[b] >= xlo) & (xsmin[b] <= xhi)
                            & (ysmax[b] >= ylo) & (ysmin[b] <= yhi))[0]
            ent = {"b": b, "t": t, "j0": j0, "i0": i0,
                   2: (np.empty(0, np.int64), np.empty((0, 1), np.int64)),
                   3: (np.empty(0, np.int64), np.empty((0, 2), np.int64)),
                   4: (np.empty(0, np.int64), np.empty((0, 3), np.int64))}
            if len(cand):
                corners = np.array(
                    [[j0, i0, 1], [j0 + _TJ - 1, i0, 1],
                     [j0, i0 + _TI - 1, 1], [j0 + _TJ - 1, i0 + _TI - 1, 1]],
                    np.float64)
                se = np.stack([(_OFF - corners @ P[k][:, cand]) / _S
                               for k in range(3)])  # [3, 4, n]
                sat_out = (se <= -_EPS_SAT).all(axis=1).any(axis=0)
                binding = se.min(axis=1) < _EPS_BIND  # [3, n]
                covers = (~binding).all(axis=0) & ~sat_out
                Wc = corners @ P[3][:, cand]
                zlo = np.maximum(Wc.min(0), zmin_tri[b][cand])
                bound = (Wc.max(0)[covers].min() + _BOUND_MARGIN) if covers.any() else np.inf
                keep = ~sat_out & (zlo <= bound)
                kidx = np.where(keep)[0]
                nb = binding[:, kidx].sum(axis=0)
                for cls, nbv in ((2, (0, 1)), (3, (2,)), (4, (3,))):
                    m = np.isin(nb, nbv)
                    ci = kidx[m]
                    edges = np.full((len(ci), cls - 1), -1, np.int64)
                    for r, cix in enumerate(ci):
                        eks = np.where(binding[:, cix])[0]
                        edges[r, :len(eks)] = eks
                    ent[cls] = (cand[ci], edges)
            tiles.append(ent)
    return tiles, planes


def _schedule(L):
    """Build shared group structure + static engine schedule from per-class
    stream lengths L = {2: L2, 3: L3, 4: L4}. Returns ordered group dicts."""
    raw = []
    for cls in (2, 3, 4):
        Vc = _VCLASS[cls]
        ngc = (L[cls] + Vc - 1) // Vc
        for g in range(ngc):
            V = min(Vc, L[cls] - g * Vc)
            raw.append((g / max(ngc, 1), cls, g * Vc, V))
    raw.sort()

    busy = {"act": 0.0, "pool": 0.0, "dve": 0.0, "dma": _DMA_BASE}
    groups = []
    co, ko = 0, 0
    for _, cls, soff, V in raw:
        a = cls
        best = None
        # configs: (drain, d). All merges on DVE (Pool has no elementwise
        # ISA on TRN2; only Act/DVE can read PSUM).
        cfgs = [(dr, d, "dve") for dr in ("act", "dve")
                for d in range(1, a + 1)]
        for drain, d, merge in cfgs:
            nb = dict(busy)
            if drain == "act":
                nb["act"] += d * V * _ACT_COL + _ACT_FIX
            elif drain == "dve":
                nb["dve"] += d * V * _DVE_PS + _DVE_FIX_PS
            else:
                nb["dma"] += a * V * _DMA_COL
            fp32 = drain == "dma"
            for lvl in range(1, a):
                if lvl >= d:  # operand still in PSUM -> DVE only
                    nb["dve"] += V * _DVE_PS + _DVE_FIX_PS
                elif merge == "dve":
                    nb["dve"] += V * (_DVE_PS if fp32 else _DVE_BF) + _DVE_FIX_SB
                else:
                    nb["pool"] += V * _POOL_COL + _POOL_FIX
            mx = max(nb.values())
            if best is None or mx < best[0]:
                best = (mx, d, drain, merge, nb)
        _, d, drain, merge, nb = best
        busy = nb
        groups.append({"cls": cls, "soff": soff, "V": V, "d": d,
                       "drain": drain, "merge": merge, "coff": co, "koff": ko})
        co += a * V
        ko += V
    return tuple((g["cls"], g["soff"], g["V"], g["d"], g["drain"], g["merge"],
                  g["coff"], g["koff"]) for g in groups), co, ko


def _build_nc(gkey):
    groups, CT, KT = gkey
    nc = bacc.Bacc("TRN2", target_bir_lowering=False, debug=False, num_devices=8)
    pix_d = nc.dram_tensor("pix", [9, 128], _BF16, kind="ExternalInput")
    coef_d = nc.dram_tensor("coef", [9, CT], _BF16, kind="ExternalInput")
    out_d = nc.dram_tensor("out", [128, KT], _BF16, kind="ExternalOutput")

    with tile.TileContext(nc) as tc:
        with (
            tc.tile_pool(name="const", bufs=1) as cpool,
            tc.tile_pool(name="scr", bufs=4) as spool,
            tc.tile_pool(name="ps", bufs=4, space="PSUM") as ppool,
        ):
            pixt = cpool.tile([9, 128], _BF16, name="pix")
            nc.sync.dma_start(pixt[:], pix_d.ap())
            coeft = cpool.tile([9, CT], _BF16, name="coef")
            # coef DMA: tiny first chunk (first group) so matmul 0 starts
            # early; remainder split between the sync and scalar DGEs.
            ng = len(groups)
            c1st = groups[1][6] if ng > 1 else CT
            cmid = groups[(ng + 1) // 2][6] if ng > 2 else CT
            nc.sync.dma_start(coeft[:, 0:c1st], coef_d.ap()[:, 0:c1st])
            if c1st < cmid:
                nc.sync.dma_start(coeft[:, c1st:cmid], coef_d.ap()[:, c1st:cmid])
            if cmid < CT:
                nc.scalar.dma_start(coeft[:, cmid:CT], coef_d.ap()[:, cmid:CT])
            outt = cpool.tile([128, KT], _BF16, name="out")
            nc.tensor.ldweights(pixt[:, :])

            odma = [0]

            def flush_out(k1):
                if k1 > odma[0]:
                    nc.sync.dma_start(out_d.ap()[:, odma[0]:k1], outt[:, odma[0]:k1])
                    odma[0] = k1

            for gi, (cls, soff, V, d, drain, merge, coff, koff) in enumerate(groups):
                a = cls
                aV = a * V
                ps = ppool.tile([128, 1024], _F32, tag="ps", name=f"ps{gi}")
                # bank-aligned matmul chops of [0, aV)
                c0 = 0
                while c0 < aV:
                    c1 = min(aV, (c0 // 512 + 1) * 512)
                    mm = nc.tensor.matmul(ps[:, c0:c1], pixt[:, :],
                                          coeft[:, coff + c0:coff + c1],
                                          start=True, stop=True)
                    mm.ins.ldweights = False
                    c0 = c1
                sc = spool.tile([128, 1024], _BF16, tag="sc", name=f"sc{gi}")
                if drain == "act":
                    nc.scalar.copy(sc[:, :d * V], ps[:, :d * V])
                else:
                    nc.vector.tensor_scalar(sc[:, :d * V], ps[:, :d * V],
                                            0.0, None,
                                            op0=mybir.AluOpType.add)
                cur = sc[:, 0:V]
                for lvl in range(1, a):
                    dst = outt[:, koff:koff + V]
                    in1 = (sc[:, lvl * V:(lvl + 1) * V] if lvl < d
                           else ps[:, lvl * V:(lvl + 1) * V])
                    nc.vector.tensor_tensor(dst, cur, in1,
                                            op=mybir.AluOpType.max)
                    cur = dst
                if (gi + 1) % max(1, (len(groups) + 1) // 2) == 0:
                    flush_out(koff + V)
            flush_out(KT)

    nc.compile()
    return nc


def _get_nc(gkey):
    if gkey not in _NC_CACHE:
        _NC_CACHE[gkey] = _build_nc(gkey)
    return _NC_CACHE[gkey]


def _prepare(vertices, faces):
    tiles, planes = _tiles(vertices, faces)

    # greedy tile -> core assignment balanced by plane count
    def tplanes(ent):
        return (2 * len(ent[2][0]) + 3 * len(ent[3][0]) + 4 * len(ent[4][0]))

    order = sorted(range(len(tiles)), key=lambda i: -tplanes(tiles[i]))
    loads = [0.0] * 8
    core_tiles = [[] for _ in range(8)]
    for i in order:
        c = loads.index(min(loads))
        core_tiles[c].append(tiles[i])
        loads[c] += tplanes(tiles[i])

    # per-core class streams; runs for host unpacking
    streams = [{2: [], 3: [], 4: []} for _ in range(8)]  # (b_idx, cand, edges, j0, i0)
    runs = [[] for _ in range(8)]  # (cls, spos, n, b, t)
    for c in range(8):
        for ent in core_tiles[c]:
            for cls in (2, 3, 4):
                ci, edges = ent[cls]
                if len(ci) == 0:
                    continue
                runs[c].append((cls, len(streams[c][cls]), len(ci),
                                ent["b"], ent["t"]))
                for r in range(len(ci)):
                    streams[c][cls].append((ent["b"], ci[r], edges[r],
                                            ent["j0"], ent["i0"]))

    L = {cls: max(len(streams[c][cls]) for c in range(8)) for cls in (2, 3, 4)}
    gkey = _schedule(L)
    groups, CT, KT = gkey

    # vectorized coef construction per core
    in_maps = []
    dj = (np.arange(128) % _TJ).astype(np.float64)
    di = (np.arange(128) // _TJ).astype(np.float64)
    pix9 = np.zeros((9, 128), _BF16_NP)
    for r in range(3):
        pix9[3 * r + 0] = dj.astype(_BF16_NP)
        pix9[3 * r + 1] = di.astype(_BF16_NP)
        pix9[3 * r + 2] = 1.0
    for c in range(8):
        coef = np.zeros((9, CT), _BF16_NP)
        for cls in (2, 3, 4):
            st = streams[c][cls]
            n = len(st)
            if n == 0:
                continue
            bv = np.array([s[0] for s in st])
            cv = np.array([s[1] for s in st])
            ev = np.array([s[2] for s in st])  # [n, cls-1]
            j0v = np.array([s[3] for s in st], np.float64)
            i0v = np.array([s[4] for s in st], np.float64)
            for lvl in range(cls):
                if lvl == 0:
                    sel = np.full(n, 3)
                    use = np.ones(n, bool)
                else:
                    sel = ev[:, lvl - 1]
                    use = sel >= 0
                    sel = np.where(use, sel, 0)
                al = planes[bv, sel, 0, cv]
                be = planes[bv, sel, 1, cv]
                ga = planes[bv, sel, 2, cv] + al * j0v + be * i0v
                al = np.where(use, al, 0.0)
                be = np.where(use, be, 0.0)
                ga = np.where(use, ga, -_BIGC)
                block = np.empty((9, n), _BF16_NP)
                h_a, m_a, l_a = _split3(al)
                h_b, m_b, l_b = _split3(be)
                h_c, m_c, l_c = _split3(ga)
                block[0], block[1], block[2] = h_a, h_b, h_c
                block[3], block[4], block[5] = m_a, m_b, m_c
                block[6], block[7], block[8] = l_a, l_b, l_c
                # scatter into groups of this class
                for (gcls, soff, V, d, drain, merge, coff, koff) in groups:
                    if gcls != cls:
                        continue
                    s0, s1 = soff, min(soff + V, n)
                    if s1 <= s0:
                        continue
                    coef[:, coff + lvl * V + 0: coff + lvl * V + (s1 - s0)] = \
                        block[:, s0:s1]
        in_maps.append({"coef": coef, "pix": pix9})

    return gkey, in_maps, runs


def _unpack(gkey, results, runs):
    groups, CT, KT = gkey
    # per class: list of (soff, V, koff) for stream-pos -> out-col mapping
    gmap = {2: [], 3: [], 4: []}
    for (cls, soff, V, d, drain, merge, coff, koff) in groups:
        gmap[cls].append((soff, V, koff))
    ntj = _W // _TJ
    out = np.full((_B, _H, _W), _CLAMP, np.float32)
    for c in range(8):
        key = results[c]["out"].astype(np.float32)  # [128, KT]
        for (cls, spos, n, b, t) in runs[c]:
            tj, ti = t % ntj, t // ntj
            j0, i0 = tj * _TJ, ti * _TI
            vals = None
            s0 = spos
            while s0 < spos + n:
                for (soff, V, koff) in gmap[cls]:
                    if soff <= s0 < soff + V:
                        s1 = min(spos + n, soff + V)
                        seg = key[:, koff + (s0 - soff): koff + (s1 - soff)]
                        m = seg.min(axis=1)
                        vals = m if vals is None else np.minimum(vals, m)
                        s0 = s1
                        break
                else:
                    raise AssertionError((cls, s0))
            blk = vals.reshape(_TI, _TJ)
            np.minimum(out[b, i0:i0 + _TI, j0:j0 + _TJ], blk,
                       out=out[b, i0:i0 + _TI, j0:j0 + _TJ])
    return out


def kernel(vertices, faces):
    vertices = np.asarray(vertices)
    faces = np.asarray(faces)
    gkey, in_maps, runs = _prepare(vertices, faces)
    nc = _get_nc(gkey)
    kw = dict(PROFILE.get("run_kwargs", {}))
    res = run_bass_kernel_spmd(nc, in_maps, list(range(8)), **kw)
    PROFILE["last_result"] = res
    return _unpack(gkey, res.results, runs)
